# revision 37
# baseline (speedup 1.0000x reference)
"""Trainium2 Bass kernel for nn_AttentionBlock (GroupNorm + single-head spatial
self-attention + residual), data-parallel over batch across 8 NeuronCores.

Reference per sample (C=256, H=W=32, N=H*W=1024 tokens, 32 groups):
    q = GN_q(x) @ Wq + bq ; k = GN_k(x) @ Wk + bk ; v = GN_v(x) @ Wv + bv
    att = softmax((q^T k) / sqrt(C)) over keys;  out = x + (att @ v^T) @ Wo + bo

Device algorithm (per core: 4 samples):
  - GroupNorm affine folds into the projection weights on the host, so the
    device only normalizes:  xh = (x - mu_g) * rsqrt(var_g + eps).
    Per-channel stats via bn_stats.  The cross-partition group combine runs
    through tiny partition-gather/broadcast DMAs (pipelined samples) or tiny
    PE matmuls (prologue, where PE is idle).  rstd = exp(-0.5*ln(var+eps))
    keeps every ScalarE op inside the one natural_log_exp_and_others table
    set (zero table reloads).
  - Scores fold Q and K into ONE matmul chain: with M^T = Wq_eff @ Wk_eff^T
    (host, includes 1/sqrt(C)), U = M @ xh (+ w1), and
    s_T[m, n] = sum_c xh[c, m] * U'[c, n].  Score terms constant along the
    softmax (key) axis cancel, so bk never appears; the bq cross term is w1,
    folded into U's eviction bias.
  - All attention matmuls run in fp8e4m3 with perf_mode=DoubleRow: operands
    carry the 256-deep contraction as value-pairs per partition
    ([128, 2, free] APs), doubling TensorE throughput.  fp8 scale factors
    (SM on the score chain, SV on the value chain) dodge fp8 denormals and
    are unwound in the exp scale and the hout eviction.
  - Softmax denominator: an all-ones stationary matmul accumulates column
    sums of e_T broadcast to every partition; 1/colsum = exp(-ln()) on
    ScalarE (DVE's RECIPROCAL is ~8x slower).  exp + all PSUM evictions are
    single [128,1024] ops over two-bank PSUM tiles.
  - hout[c, n] = V^T e_T; output projection (bf16) eviction fuses
    (+bo, +x residual) in one scalar_tensor_tensor.
  Measured end-to-end scale-relative absmax error vs the f32 reference
  is ~2e-4 (fp8) / ~1.4e-5 (bf16 variant).
"""

import numpy as np
import ml_dtypes

import concourse.bass as bass
import concourse.tile as tile
from concourse import mybir
from concourse.vector_clock import ScopedClock

F32 = mybir.dt.float32
BF16 = mybir.dt.bfloat16
FP8 = mybir.dt.float8e4
AF = mybir.ActivationFunctionType
ALU = mybir.AluOpType
DR = mybir.MatmulPerfMode.DoubleRow

B, C, H, W = 32, 256, 32, 32
N = H * W            # 1024 spatial tokens
G = 32               # groups
GS = C // G          # 8 channels per group
EPS = 1e-5
NCORES = 8
BS = B // NCORES     # 4 samples per core
CT = C // 128        # 2 channel partition-tiles
MT = N // 128        # 8 token partition-tiles
SM = 256.0           # fp8 scale on the score chain (M, U)
SV = 32.0           # fp8 scale on the value chain (Wv, V); |v*SV| < ~150


def _patch_tile_drain():
    """walrus in this container allows only ONE sync wait per instruction;
    Tile's final drain carries one wait per live logical processor.  Split
    the waits across SP nops."""
    if getattr(tile.TileContext, "_drain_patched", False):
        return

    def _drain_and_barrier(self, tick_clock, wait_clock):
        nc = self.nc
        drain_inst = nc.sync.drain()
        wait_clock.add_sem_waits(
            drain_inst.ins, ScopedClock({None: tick_clock.global_clock})
        )
        si = drain_inst.ins.sync_info
        waits = list(si.on_wait or [])
        if len(waits) > 1:
            si.on_wait = waits[:1]
            engs = [nc.vector, nc.scalar, nc.tensor, nc.gpsimd, nc.sync]
            for idx, w in enumerate(waits[1:]):
                name = getattr(w, "ant_name", "") or ""
                if name.startswith("DVE"):
                    eng = nc.vector
                elif name.startswith("Activation"):
                    eng = nc.scalar
                elif name.startswith("PE"):
                    eng = nc.tensor
                elif name.startswith("Pool"):
                    eng = nc.gpsimd
                elif name.startswith(("SP", "DMA")):
                    eng = nc.sync
                else:
                    eng = engs[idx % len(engs)]
                nop_inst = eng.nop()
                nop_inst.ins.sync_info = mybir.SyncInfo(on_wait=[w], on_update=[])

        nc.all_engine_barrier()
        assert self.sems is not None
        popped = nc._tile_sem_poison_stack.pop()
        assert popped is self._sem_poison
        nc.clear_and_free_semaphores(list(self.sems.allocated().values()))
        nc.all_engine_barrier()

    tile.TileContext._drain_and_barrier = _drain_and_barrier
    tile.TileContext._drain_patched = True


def _split_multi_waits(nc):
    """Hoist extra sync waits onto same-engine nops placed just before the
    instruction (engines execute their stream in order, so this is
    equivalent); walrus supports a single wait slot per instruction."""
    k = [0]
    for f in nc.m.functions:
        for b in f.blocks:
            insts = list(b.instructions)
            out = []
            changed = False
            for inst in insts:
                si = inst.sync_info
                if si is not None and si.on_wait and len(si.on_wait) > 1:
                    waits = list(si.on_wait)
                    for w in waits[:-1]:
                        nop = mybir.InstNoOp(
                            name=f"waitsplit-{k[0]}", ins=[], outs=[])
                        k[0] += 1
                        nop.engine = inst.engine
                        nop.sync_info = mybir.SyncInfo(
                            on_wait=[w], on_update=[])
                        out.append(nop)
                        nc.register_instruction(nop, overwrite=True)
                    si.on_wait = waits[-1:]
                    changed = True
                out.append(inst)
            if changed:
                lst = b.instructions
                lst.clear()
                lst.extend(out)
    return nc


def build_nc():
    _patch_tile_drain()
    nc = bass.Bass(trn_type="TRN2")

    x_d = nc.dram_tensor("x", [BS, C, N], F32, kind="ExternalInput")
    y_d = nc.dram_tensor("y", [BS, C, N], F32, kind="ExternalOutput")
    mt_d = nc.dram_tensor("mt", [128, 2, C], FP8, kind="ExternalInput")
    wv_d = nc.dram_tensor("wv", [128, 2, C], FP8, kind="ExternalInput")
    wo_d = nc.dram_tensor("wo", [CT, 128, C], BF16, kind="ExternalInput")
    w1_d = nc.dram_tensor("w1", [CT, 128, 1], F32, kind="ExternalInput")
    bo_d = nc.dram_tensor("bo", [CT, 128, 1], F32, kind="ExternalInput")
    bv_d = nc.dram_tensor("bv_bc", [128, 2 * C], F32, kind="ExternalInput")
    ag_d = nc.dram_tensor("ag", [CT, 128, G], F32, kind="ExternalInput")
    bg_d = nc.dram_tensor("bg", [CT, G, 128], F32, kind="ExternalInput")

    with tile.TileContext(nc) as tc:
        _emit(nc, tc, x_d, y_d, mt_d, wv_d, wo_d, w1_d, bo_d, bv_d,
              ag_d, bg_d)
    _split_multi_waits(nc)
    return nc


def _emit(nc, tc, x_d, y_d, mt_d, wv_d, wo_d, w1_d, bo_d, bv_d, ag_d, bg_d):
    from contextlib import ExitStack
    import dataclasses
    ctx = ExitStack()
    with ctx:
        singles = ctx.enter_context(tc.tile_pool(name="singles", bufs=1))
        xpool = ctx.enter_context(tc.tile_pool(name="x", bufs=3))
        xhpool = ctx.enter_context(tc.tile_pool(name="xh", bufs=2))
        stpool = ctx.enter_context(tc.tile_pool(name="st", bufs=2))
        upool = ctx.enter_context(tc.tile_pool(name="u", bufs=2))
        vpool = ctx.enter_context(tc.tile_pool(name="v", bufs=2))
        epool = ctx.enter_context(tc.tile_pool(name="e", bufs=2))
        hpool = ctx.enter_context(tc.tile_pool(name="h", bufs=2))
        opool = ctx.enter_context(tc.tile_pool(name="o", bufs=2))
        ppb = ctx.enter_context(tc.tile_pool(name="psb", bufs=4, space="PSUM"))

        # ---- warm the ACT table (ln/exp set) while the first DMAs run ----
        eps_sb = singles.tile([128, 1], F32, tag="eps", name="eps")
        nc.vector.memset(eps_sb[:], EPS)
        actwarm = singles.tile([128, 1], F32, tag="actwarm", name="actwarm")
        nc.scalar.activation(actwarm[:], eps_sb[:], AF.Exp)
        nc.scalar.activation(actwarm[:], actwarm[:], AF.Ln)

        x_sb = [None] * BS
        xh8 = [None] * BS    # [128, 2, N] fp8 pair layout: c = 128j + p
        u8 = [None] * BS     # [128, 2, N] fp8 (score-chain, scaled by SM)
        v8 = [None] * BS     # 4x [128, 2, C] fp8 (value chain, scaled by SV)

        def emit_load_x(s, spread=False):
            # split across both DMA queues and in halves, so bn_stats can
            # start as soon as data streams in.  spread=True (prologue)
            # splits each tile across BOTH queues for minimum latency.
            x_sb[s] = [xpool.tile([128, N], F32, tag=f"x{t}", name=f"x{t}")
                       for t in range(CT)]
            for t in range(CT):
                for h in range(2):
                    if spread:
                        eng = nc.sync if h == 0 else nc.gpsimd
                    else:
                        eng = nc.sync if t == 0 else nc.gpsimd
                    eng.dma_start(
                        x_sb[s][t][:, h * 512:(h + 1) * 512],
                        x_d[s, t * 128:(t + 1) * 128,
                            h * 512:(h + 1) * 512])

        emit_load_x(0)

        # ---- constants / weights ----
        mt_sb = singles.tile([128, 2, C], FP8, tag="mt", name="mt")
        wv_sb = singles.tile([128, 2, C], FP8, tag="wv", name="wv")
        wo_sb = [singles.tile([128, C], BF16, tag=f"wo{t}", name=f"wo{t}")
                 for t in range(CT)]
        w1_sb = [singles.tile([128, 1], F32, tag=f"w1{t}", name=f"w1{t}")
                 for t in range(CT)]
        bo_sb = [singles.tile([128, 1], F32, tag=f"bo{t}", name=f"bo{t}")
                 for t in range(CT)]
        bv_sb = singles.tile([128, 2 * C], F32, tag="bvbc", name="bvbc")
        ag_sb = [singles.tile([128, G], F32, tag=f"ag{t}", name=f"ag{t}")
                 for t in range(CT)]
        bg_sb = [singles.tile([G, 128], F32, tag=f"bg{t}", name=f"bg{t}")
                 for t in range(CT)]
        nc.gpsimd.dma_start(mt_sb[:], mt_d[:, :, :])
        nc.gpsimd.dma_start(wv_sb[:], wv_d[:, :, :])
        for t in range(CT):
            nc.gpsimd.dma_start(wo_sb[t][:], wo_d[t])
            nc.gpsimd.dma_start(w1_sb[t][:], w1_d[t])
            nc.gpsimd.dma_start(bo_sb[t][:], bo_d[t])
            nc.sync.dma_start(ag_sb[t][:], ag_d[t])
            nc.sync.dma_start(bg_sb[t][:], bg_d[t])
        nc.gpsimd.dma_start(bv_sb[:], bv_d[:, :])
        ones_sb = singles.tile([128, 2, 128], FP8, tag="ones", name="ones")
        nc.vector.memset(ones_sb[:], 1.0)

        gn_stats = [None] * BS


        def emit_gn_stats(s):
            # per-channel stats on DVE only
            stats2 = []
            for t in range(CT):
                st6 = stpool.tile([128, 2, 6], F32, tag=f"st6_{t}",
                                  name=f"st6_{t}")
                for half in range(2):
                    nc.vector.bn_stats(
                        out=st6[:, half, :],
                        in_=x_sb[s][t][:, half * 512:(half + 1) * 512],
                    )
                aggr = stpool.tile([128, 2], F32, tag=f"aggr{t}",
                                   name=f"aggr{t}")
                nc.vector.bn_aggr(out=aggr[:], in_=st6[:])
                st2 = stpool.tile([128, 2], F32, tag=f"st2_{t}",
                                  name=f"st2_{t}")
                nc.vector.tensor_copy(st2[:, 0:1], aggr[:, 0:1])
                # msq = mean*mean + var
                nc.vector.tensor_scalar(
                    out=st2[:, 1:2], in0=aggr[:, 0:1],
                    scalar1=aggr[:, 0:1], scalar2=aggr[:, 1:2],
                    op0=ALU.mult, op1=ALU.add,
                )
                stats2.append(st2)
            gn_stats[s] = stats2

        def alloc_xh8(s):
            xh8[s] = xhpool.tile([128, 2, N], FP8, tag="xh8", name="xh8")

        def emit_gn_combine(s):
            # group combine on PE (tiny matmul; placed right after AV in the
            # PE stream so its DVE/ACT middle hides under the AV matmuls),
            # then group-level mu/rstd on 32 partitions
            stats2 = gn_stats[s]
            gps = ppb.tile([G, 2], F32, tag="big", name="gps")
            for t in range(CT):
                nc.tensor.matmul(gps[:], ag_sb[t][:], stats2[t][:],
                                 start=(t == 0), stop=(t == CT - 1))
            g2 = stpool.tile([G, 2], F32, tag="g2", name="g2")
            nc.vector.tensor_copy(g2[:], gps[:])
            murs = stpool.tile([G, 2], F32, tag="murs", name="murs")
            nc.vector.tensor_copy(murs[:, 0:1], g2[:, 0:1])
            nv = stpool.tile([G, 1], F32, tag="nv", name="nv")
            nc.vector.tensor_scalar(
                out=nv[:], in0=g2[:, 0:1],
                scalar1=g2[:, 0:1], scalar2=g2[:, 1:2],
                op0=ALU.mult, op1=ALU.subtract)
            lnv = stpool.tile([G, 1], F32, tag="lnv", name="lnv")
            nc.scalar.activation(lnv[:], nv[:], AF.Ln,
                                 bias=eps_sb[0:G, :], scale=-1.0)
            nc.scalar.activation(murs[:, 1:2], lnv[:], AF.Exp, scale=-0.5)
            return murs

        def emit_gn_bcast(s, murs):
            # broadcast group (mu, rstd) back to channel partitions with a
            # tiny PE matmul, then xhat on DVE
            alloc_xh8(s)
            for t in range(CT):
                bcps = ppb.tile([128, 2], F32, tag="big", name="bcps")
                nc.tensor.matmul(bcps[:], bg_sb[t][:], murs[:],
                                 start=True, stop=True)
                mubc = stpool.tile([128, 2], F32, tag=f"mubc{t}",
                                   name=f"mubc{t}")
                nc.vector.tensor_copy(mubc[:], bcps[:])
                nc.vector.tensor_scalar(
                    out=xh8[s][:, t, :], in0=x_sb[s][t][:],
                    scalar1=mubc[:, 0:1], scalar2=mubc[:, 1:2],
                    op0=ALU.subtract, op1=ALU.mult,
                )

        def emit_gn_finish_pe(s):
            # prologue-only variant: group combine via PE matmuls (PE is
            # idle before the first projection; ~4us shorter than the DMA
            # round-trips)
            stats2 = gn_stats[s]
            gps = ppb.tile([G, 2], F32, tag="big", name="gps")
            for t in range(CT):
                nc.tensor.matmul(gps[:], ag_sb[t][:], stats2[t][:],
                                 start=(t == 0), stop=(t == CT - 1))
            gst = stpool.tile([G, 2], F32, tag="gst", name="gst")
            nc.vector.tensor_copy(gst[:], gps[:])
            alloc_xh8(s)
            for t in range(CT):
                bcps = ppb.tile([128, 2], F32, tag="big", name="bcps")
                nc.tensor.matmul(bcps[:], bg_sb[t][:], gst[:],
                                 start=True, stop=True)
                mm = stpool.tile([128, 2], F32, tag=f"mm{t}", name=f"mm{t}")
                nc.vector.tensor_copy(mm[:], bcps[:])
                nv = stpool.tile([128, 1], F32, tag=f"nv{t}", name=f"nv{t}")
                nc.vector.tensor_scalar(
                    out=nv[:], in0=mm[:, 0:1],
                    scalar1=mm[:, 0:1], scalar2=mm[:, 1:2],
                    op0=ALU.mult, op1=ALU.subtract)
                lnv = stpool.tile([128, 1], F32, tag=f"lnv{t}",
                                  name=f"lnv{t}")
                nc.scalar.activation(lnv[:], nv[:], AF.Ln,
                                     bias=eps_sb[:], scale=-1.0)
                rs = stpool.tile([128, 1], F32, tag=f"rs{t}", name=f"rs{t}")
                nc.scalar.activation(rs[:], lnv[:], AF.Exp, scale=-0.5)
                nc.vector.tensor_scalar(
                    out=xh8[s][:, t, :], in0=x_sb[s][t][:],
                    scalar1=mm[:, 0:1], scalar2=rs[:],
                    op0=ALU.subtract, op1=ALU.mult,
                )

        def emit_uv(s):
            # U' = (M @ xh)*SM + w1*SM  in fp8 pair layout [128, 2, N];
            # V = (xh^T Wv + bv)*SV     in fp8 pair tiles [128, 2, C]
            u8[s] = upool.tile([128, 2, N], FP8, tag="u8", name="u8")
            for ct in range(CT):
                ps = ppb.tile([128, N], F32, tag="big", name="ps")
                for nch in range(2):
                    nc.tensor.matmul(
                        ps[:, nch * 512:(nch + 1) * 512],
                        mt_sb[:, :, ct * 128:(ct + 1) * 128],
                        xh8[s][:, :, nch * 512:(nch + 1) * 512],
                        start=True, stop=True, perf_mode=DR)
                nc.scalar.activation(
                    u8[s][:, ct, :], ps[:], AF.Identity, bias=w1_sb[ct][:])
            v8[s] = [vpool.tile([128, 2, C], FP8, tag=f"v8_{m2}",
                                name=f"v8_{m2}") for m2 in range(MT // 2)]
            for m2 in range(MT // 2):
                ps = ppb.tile([128, 2 * C], F32, tag="big", name="psv")
                for j in range(2):
                    nc.tensor.matmul(
                        ps[:, j * C:(j + 1) * C],
                        xh8[s][:, :, (2 * m2 + j) * 128:(2 * m2 + j + 1) * 128],
                        wv_sb[:],
                        start=True, stop=True, perf_mode=DR)
                nc.vector.tensor_tensor(
                    out=v8[s][m2][:].rearrange("p a b -> p (a b)"),
                    in0=ps[:], in1=bv_sb[:], op=ALU.add)

        # ---- prologue (x(0) DMA already issued at the very top) ----
        emit_load_x(1)
        emit_gn_stats(0)
        emit_gn_finish_pe(0)
        emit_uv(0)

        def emit_scores(s):
            # scores (transposed, fp8 DoubleRow) + exp -> e8 pair tiles
            e8[s] = [epool.tile([128, 2, N], FP8, tag=f"e8_{m2}",
                                name=f"e8_{m2}") for m2 in range(MT // 2)]
            for mt in range(MT):
                ps = ppb.tile([128, N], F32, tag="big", name="ps")
                for nch in range(2):
                    nc.tensor.matmul(
                        ps[:, nch * 512:(nch + 1) * 512],
                        xh8[s][:, :, mt * 128:(mt + 1) * 128],
                        u8[s][:, :, nch * 512:(nch + 1) * 512],
                        start=True, stop=True, perf_mode=DR)
                nc.scalar.activation(e8[s][mt // 2][:, mt % 2, :], ps[:],
                                     AF.Exp, scale=1.0 / SM)

        e8 = [None] * BS
        emit_scores(0)

        for s in range(BS):
            if s + 1 < BS:
                emit_gn_stats(s + 1)

            # --- colsum (ones lhsT -> broadcast) + hout accumulation ---
            cp = ppb.tile([128, N], F32, tag="big", name="cp")
            hp = [ppb.tile([128, N], F32, tag="big", name="hp")
                  for _ in range(CT)]
            M2 = MT // 2
            for m2 in range(M2):
                for nch in range(2):
                    nc.tensor.matmul(
                        cp[:, nch * 512:(nch + 1) * 512],
                        ones_sb[:],
                        e8[s][m2][:, :, nch * 512:(nch + 1) * 512],
                        start=(m2 == 0), stop=(m2 == M2 - 1),
                        perf_mode=DR)
                for ch in range(CT):
                    for nch in range(2):
                        nc.tensor.matmul(
                            hp[ch][:, nch * 512:(nch + 1) * 512],
                            v8[s][m2][:, :, ch * 128:(ch + 1) * 128],
                            e8[s][m2][:, :, nch * 512:(nch + 1) * 512],
                            start=(m2 == 0), stop=(m2 == M2 - 1),
                            perf_mode=DR)

            # next sample's groupnorm tail + projections keep PE busy while
            # this sample's reciprocal/eviction chain runs on ScalarE/DVE
            if s + 1 < BS:
                murs_next = emit_gn_combine(s + 1)
                emit_gn_bcast(s + 1, murs_next)
                emit_uv(s + 1)

            # --- 1/colsum via exp(-ln()) on ScalarE ---
            lncs = hpool.tile([128, N], F32, tag="lncs", name="lncs")
            rbc = hpool.tile([128, N], F32, tag="rbc", name="rbc")
            nc.scalar.activation(lncs[:], cp[:], AF.Ln)
            nc.scalar.activation(rbc[:], lncs[:], AF.Exp, scale=-1.0)
            # --- hout eviction: unwind SV and normalize; halves first so
            # the output projection can start after two of the four ops ---
            h_sb = [hpool.tile([128, N], BF16, tag=f"h{ch}", name=f"h{ch}")
                    for ch in range(CT)]
            for nch in range(2):
                for ch in range(CT):
                    sl = slice(nch * 512, (nch + 1) * 512)
                    nc.vector.scalar_tensor_tensor(
                        out=h_sb[ch][:, sl], in0=hp[ch][:, sl],
                        scalar=1.0 / SV,
                        in1=rbc[:, sl], op0=ALU.mult, op1=ALU.mult)

            if s + 1 < BS:
                emit_scores(s + 1)

            # --- output projection (bf16) + bias + residual ---
            o_sb = [opool.tile([128, N], F32, tag=f"o{dt}", name=f"o{dt}")
                    for dt in range(CT)]
            fps = [ppb.tile([128, N], F32, tag="big", name="ps")
                   for _ in range(CT)]
            for nch in range(2):
                for dt in range(CT):
                    for ct in range(CT):
                        nc.tensor.matmul(
                            fps[dt][:, nch * 512:(nch + 1) * 512],
                            wo_sb[ct][:, dt * 128:(dt + 1) * 128],
                            h_sb[ct][:, nch * 512:(nch + 1) * 512],
                            start=(ct == 0), stop=(ct == CT - 1))
            for dt in range(CT):
                # out = (psum + bo) + x  in one DVE pass
                nc.vector.scalar_tensor_tensor(
                    out=o_sb[dt][:],
                    in0=fps[dt][:],
                    scalar=bo_sb[dt][:],
                    in1=x_sb[s][dt][:],
                    op0=ALU.add, op1=ALU.add)
            for dt in range(CT):
                eng = nc.sync if dt == 0 else nc.gpsimd
                eng.dma_start(y_d[s, dt * 128:(dt + 1) * 128, :],
                              o_sb[dt][:])
            if s + 2 < BS:
                emit_load_x(s + 2)


_NC_CACHE = {}


def _get_nc():
    if "nc" not in _NC_CACHE:
        _NC_CACHE["nc"] = build_nc()
    return _NC_CACHE["nc"]


def _pair(a):
    """[C, X] -> [128, 2, X] fp8 pair layout with c = 128*j + p."""
    a = np.asarray(a, np.float32)
    return np.ascontiguousarray(
        a.reshape(2, 128, a.shape[1]).transpose(1, 0, 2))


def _fp8(a):
    return np.clip(np.asarray(a, np.float32),
                   -240, 240).astype(ml_dtypes.float8_e4m3)


def make_in_maps(**inputs):
    f32 = np.float32
    x = np.asarray(inputs["x"], f32).reshape(B, C, N)
    Wq = np.asarray(inputs["Wq"], f32)
    Wk = np.asarray(inputs["Wk"], f32)
    Wv = np.asarray(inputs["Wv"], f32)
    Wo = np.asarray(inputs["Wo"], f32)
    bq = np.asarray(inputs["bq"], f32)
    bv = np.asarray(inputs["bv"], f32)
    bo = np.asarray(inputs["bo"], f32)
    gq_s = np.asarray(inputs["gq_s"], f32)
    gq_b = np.asarray(inputs["gq_b"], f32)
    gk_s = np.asarray(inputs["gk_s"], f32)
    gv_s = np.asarray(inputs["gv_s"], f32)
    gv_b = np.asarray(inputs["gv_b"], f32)
    # bk and gk_b only shift scores uniformly along the softmax axis -> cancel

    inv_sqrt_c = float(C) ** -0.5
    Wq_eff = (gq_s[:, None] * Wq) * inv_sqrt_c
    bq_eff = (gq_b @ Wq + bq) * inv_sqrt_c
    Wk_eff = gk_s[:, None] * Wk
    m_t = (Wq_eff @ Wk_eff.T) * SM       # lhsT for U: [c', c], fp8-scaled
    w1 = (Wk_eff @ bq_eff) * SM          # [c]
    Wv_eff = gv_s[:, None] * Wv
    bv_eff = gv_b @ Wv + bv

    bf = ml_dtypes.bfloat16
    ag = np.zeros((C, G), f32)
    bg = np.zeros((G, C), f32)
    for c in range(C):
        ag[c, c // GS] = 1.0 / GS
        bg[c // GS, c] = 1.0

    shared = {
        "mt": _fp8(_pair(m_t)),
        "wv": _fp8(_pair(Wv_eff * SV)),
        "wo": Wo.astype(bf).reshape(CT, 128, C),
        "w1": w1.astype(f32).reshape(CT, 128, 1),
        "bo": bo.reshape(CT, 128, 1),
        "bv_bc": np.tile(bv_eff[None, :] * SV, (128, 2)).astype(f32),
        "ag": np.ascontiguousarray(ag.reshape(CT, 128, G)),
        "bg": np.ascontiguousarray(bg.reshape(G, CT, 128).transpose(1, 0, 2)),
    }
    in_maps = []
    for i in range(NCORES):
        m = dict(shared)
        m["x"] = np.ascontiguousarray(x[i * BS:(i + 1) * BS])
        in_maps.append(m)
    return in_maps


def run_sharded(inputs, trace=False, **kwargs):
    from concourse.bass_utils import run_bass_kernel_spmd
    nc = _get_nc()
    in_maps = make_in_maps(**inputs)
    res = run_bass_kernel_spmd(nc, in_maps, core_ids=list(range(NCORES)),
                               trace=trace, **kwargs)
    outs = [np.asarray(res.results[i]["y"], np.float32) for i in range(NCORES)]
    full = np.concatenate(outs, axis=0).reshape(B, C, H, W)
    return full, res


def kernel(**inputs):
    out, _ = run_sharded(inputs, trace=False)
    return out


# revision 38
# speedup vs baseline: 1.0172x; 1.0172x over previous
"""Trainium2 Bass kernel for nn_AttentionBlock (GroupNorm + single-head spatial
self-attention + residual), data-parallel over batch across 8 NeuronCores.

Reference per sample (C=256, H=W=32, N=H*W=1024 tokens, 32 groups):
    q = GN_q(x) @ Wq + bq ; k = GN_k(x) @ Wk + bk ; v = GN_v(x) @ Wv + bv
    att = softmax((q^T k) / sqrt(C)) over keys;  out = x + (att @ v^T) @ Wo + bo

Device algorithm (per core: 4 samples):
  - GroupNorm affine folds into the projection weights on the host, so the
    device only normalizes:  xh = (x - mu_g) * rsqrt(var_g + eps).
    Per-channel stats via bn_stats.  The cross-partition group combine runs
    through tiny partition-gather/broadcast DMAs (pipelined samples) or tiny
    PE matmuls (prologue, where PE is idle).  rstd = exp(-0.5*ln(var+eps))
    keeps every ScalarE op inside the one natural_log_exp_and_others table
    set (zero table reloads).
  - Scores fold Q and K into ONE matmul chain: with M^T = Wq_eff @ Wk_eff^T
    (host, includes 1/sqrt(C)), U = M @ xh (+ w1), and
    s_T[m, n] = sum_c xh[c, m] * U'[c, n].  Score terms constant along the
    softmax (key) axis cancel, so bk never appears; the bq cross term is w1,
    folded into U's eviction bias.
  - All attention matmuls run in fp8e4m3 with perf_mode=DoubleRow: operands
    carry the 256-deep contraction as value-pairs per partition
    ([128, 2, free] APs), doubling TensorE throughput.  fp8 scale factors
    (SM on the score chain, SV on the value chain) dodge fp8 denormals and
    are unwound in the exp scale and the hout eviction.
  - Softmax denominator: an all-ones stationary matmul accumulates column
    sums of e_T broadcast to every partition; 1/colsum = exp(-ln()) on
    ScalarE (DVE's RECIPROCAL is ~8x slower).  exp + all PSUM evictions are
    single [128,1024] ops over two-bank PSUM tiles.
  - hout[c, n] = V^T e_T; output projection (bf16) eviction fuses
    (+bo, +x residual) in one scalar_tensor_tensor.
  Measured end-to-end scale-relative absmax error vs the f32 reference
  is ~2e-4 (fp8) / ~1.4e-5 (bf16 variant).
"""

import numpy as np
import ml_dtypes

import concourse.bass as bass
import concourse.tile as tile
from concourse import mybir
from concourse.vector_clock import ScopedClock

F32 = mybir.dt.float32
BF16 = mybir.dt.bfloat16
FP8 = mybir.dt.float8e4
AF = mybir.ActivationFunctionType
ALU = mybir.AluOpType
DR = mybir.MatmulPerfMode.DoubleRow

B, C, H, W = 32, 256, 32, 32
N = H * W            # 1024 spatial tokens
G = 32               # groups
GS = C // G          # 8 channels per group
EPS = 1e-5
NCORES = 8
BS = B // NCORES     # 4 samples per core
CT = C // 128        # 2 channel partition-tiles
MT = N // 128        # 8 token partition-tiles
SM = 256.0           # fp8 scale on the score chain (M, U)
SV = 32.0           # fp8 scale on the value chain (Wv, V); |v*SV| < ~150


def _patch_tile_drain():
    """walrus in this container allows only ONE sync wait per instruction;
    Tile's final drain carries one wait per live logical processor.  Split
    the waits across SP nops."""
    if getattr(tile.TileContext, "_drain_patched", False):
        return

    def _drain_and_barrier(self, tick_clock, wait_clock):
        nc = self.nc
        drain_inst = nc.sync.drain()
        wait_clock.add_sem_waits(
            drain_inst.ins, ScopedClock({None: tick_clock.global_clock})
        )
        si = drain_inst.ins.sync_info
        waits = list(si.on_wait or [])
        if len(waits) > 1:
            si.on_wait = waits[:1]
            for w in waits[1:]:
                nop_inst = nc.sync.nop()
                nop_inst.ins.sync_info = mybir.SyncInfo(on_wait=[w], on_update=[])

        nc.all_engine_barrier()
        assert self.sems is not None
        popped = nc._tile_sem_poison_stack.pop()
        assert popped is self._sem_poison
        nc.clear_and_free_semaphores(list(self.sems.allocated().values()))
        nc.all_engine_barrier()

    tile.TileContext._drain_and_barrier = _drain_and_barrier
    tile.TileContext._drain_patched = True


def _split_multi_waits(nc):
    """Hoist extra sync waits onto same-engine nops placed just before the
    instruction (engines execute their stream in order, so this is
    equivalent); walrus supports a single wait slot per instruction."""
    k = [0]
    for f in nc.m.functions:
        for b in f.blocks:
            insts = list(b.instructions)
            out = []
            changed = False
            for inst in insts:
                si = inst.sync_info
                if si is not None and si.on_wait and len(si.on_wait) > 1:
                    waits = list(si.on_wait)
                    for w in waits[:-1]:
                        nop = mybir.InstNoOp(
                            name=f"waitsplit-{k[0]}", ins=[], outs=[])
                        k[0] += 1
                        nop.engine = inst.engine
                        nop.sync_info = mybir.SyncInfo(
                            on_wait=[w], on_update=[])
                        out.append(nop)
                        nc.register_instruction(nop, overwrite=True)
                    si.on_wait = waits[-1:]
                    changed = True
                out.append(inst)
            if changed:
                lst = b.instructions
                lst.clear()
                lst.extend(out)
    return nc


def build_nc():
    _patch_tile_drain()
    nc = bass.Bass(trn_type="TRN2")

    x_d = nc.dram_tensor("x", [BS, C, N], F32, kind="ExternalInput")
    y_d = nc.dram_tensor("y", [BS, C, N], F32, kind="ExternalOutput")
    mt_d = nc.dram_tensor("mt", [128, 2, C], FP8, kind="ExternalInput")
    wv_d = nc.dram_tensor("wv", [128, 2, C], FP8, kind="ExternalInput")
    wo_d = nc.dram_tensor("wo", [CT, 128, C], BF16, kind="ExternalInput")
    w1_d = nc.dram_tensor("w1", [CT, 128, 1], F32, kind="ExternalInput")
    bo_d = nc.dram_tensor("bo", [CT, 128, 1], F32, kind="ExternalInput")
    bv_d = nc.dram_tensor("bv_bc", [128, 2 * C], F32, kind="ExternalInput")
    ag_d = nc.dram_tensor("ag", [CT, 128, G], F32, kind="ExternalInput")
    bg_d = nc.dram_tensor("bg", [CT, G, 128], F32, kind="ExternalInput")

    with tile.TileContext(nc) as tc:
        _emit(nc, tc, x_d, y_d, mt_d, wv_d, wo_d, w1_d, bo_d, bv_d,
              ag_d, bg_d)
    _split_multi_waits(nc)
    return nc


def _emit(nc, tc, x_d, y_d, mt_d, wv_d, wo_d, w1_d, bo_d, bv_d, ag_d, bg_d):
    from contextlib import ExitStack
    import dataclasses
    ctx = ExitStack()
    with ctx:
        singles = ctx.enter_context(tc.tile_pool(name="singles", bufs=1))
        xpool = ctx.enter_context(tc.tile_pool(name="x", bufs=3))
        xhpool = ctx.enter_context(tc.tile_pool(name="xh", bufs=2))
        stpool = ctx.enter_context(tc.tile_pool(name="st", bufs=2))
        upool = ctx.enter_context(tc.tile_pool(name="u", bufs=2))
        vpool = ctx.enter_context(tc.tile_pool(name="v", bufs=2))
        epool = ctx.enter_context(tc.tile_pool(name="e", bufs=2))
        hpool = ctx.enter_context(tc.tile_pool(name="h", bufs=2))
        opool = ctx.enter_context(tc.tile_pool(name="o", bufs=2))
        ppb = ctx.enter_context(tc.tile_pool(name="psb", bufs=4, space="PSUM"))

        # ---- warm the ACT table (ln/exp set) while the first DMAs run ----
        eps_sb = singles.tile([128, 1], F32, tag="eps", name="eps")
        nc.vector.memset(eps_sb[:], EPS)
        actwarm = singles.tile([128, 1], F32, tag="actwarm", name="actwarm")
        nc.scalar.activation(actwarm[:], eps_sb[:], AF.Exp)
        nc.scalar.activation(actwarm[:], actwarm[:], AF.Ln)

        x_sb = [None] * BS
        xh8 = [None] * BS    # [128, 2, N] fp8 pair layout: c = 128j + p
        u8 = [None] * BS     # [128, 2, N] fp8 (score-chain, scaled by SM)
        v8 = [None] * BS     # 4x [128, 2, C] fp8 (value chain, scaled by SV)

        def emit_load_x(s, spread=False):
            # split across both DMA queues and in halves, so bn_stats can
            # start as soon as data streams in.  spread=True (prologue)
            # splits each tile across BOTH queues for minimum latency.
            x_sb[s] = [xpool.tile([128, N], F32, tag=f"x{t}", name=f"x{t}")
                       for t in range(CT)]
            for t in range(CT):
                for h in range(2):
                    if spread:
                        eng = nc.sync if h == 0 else nc.gpsimd
                    else:
                        eng = nc.sync if t == 0 else nc.gpsimd
                    eng.dma_start(
                        x_sb[s][t][:, h * 512:(h + 1) * 512],
                        x_d[s, t * 128:(t + 1) * 128,
                            h * 512:(h + 1) * 512])

        emit_load_x(0)

        # ---- constants / weights ----
        mt_sb = singles.tile([128, 2, C], FP8, tag="mt", name="mt")
        wv_sb = singles.tile([128, 2, C], FP8, tag="wv", name="wv")
        wo_sb = [singles.tile([128, C], BF16, tag=f"wo{t}", name=f"wo{t}")
                 for t in range(CT)]
        w1_sb = [singles.tile([128, 1], F32, tag=f"w1{t}", name=f"w1{t}")
                 for t in range(CT)]
        bo_sb = [singles.tile([128, 1], F32, tag=f"bo{t}", name=f"bo{t}")
                 for t in range(CT)]
        bv_sb = singles.tile([128, 2 * C], F32, tag="bvbc", name="bvbc")
        ag_sb = [singles.tile([128, G], F32, tag=f"ag{t}", name=f"ag{t}")
                 for t in range(CT)]
        bg_sb = [singles.tile([G, 128], F32, tag=f"bg{t}", name=f"bg{t}")
                 for t in range(CT)]
        nc.gpsimd.dma_start(mt_sb[:], mt_d[:, :, :])
        nc.gpsimd.dma_start(wv_sb[:], wv_d[:, :, :])
        for t in range(CT):
            nc.gpsimd.dma_start(wo_sb[t][:], wo_d[t])
            nc.gpsimd.dma_start(w1_sb[t][:], w1_d[t])
            nc.gpsimd.dma_start(bo_sb[t][:], bo_d[t])
            nc.sync.dma_start(ag_sb[t][:], ag_d[t])
            nc.sync.dma_start(bg_sb[t][:], bg_d[t])
        nc.gpsimd.dma_start(bv_sb[:], bv_d[:, :])
        ones_sb = singles.tile([128, 2, 128], FP8, tag="ones", name="ones")
        nc.vector.memset(ones_sb[:], 1.0)

        gn_stats = [None] * BS


        def emit_gn_stats(s):
            # per-channel stats on DVE only
            stats2 = []
            for t in range(CT):
                st6 = stpool.tile([128, 2, 6], F32, tag=f"st6_{t}",
                                  name=f"st6_{t}")
                for half in range(2):
                    nc.vector.bn_stats(
                        out=st6[:, half, :],
                        in_=x_sb[s][t][:, half * 512:(half + 1) * 512],
                    )
                aggr = stpool.tile([128, 2], F32, tag=f"aggr{t}",
                                   name=f"aggr{t}")
                nc.vector.bn_aggr(out=aggr[:], in_=st6[:])
                st2 = stpool.tile([128, 2], F32, tag=f"st2_{t}",
                                  name=f"st2_{t}")
                nc.vector.tensor_copy(st2[:, 0:1], aggr[:, 0:1])
                # msq = mean*mean + var
                nc.vector.tensor_scalar(
                    out=st2[:, 1:2], in0=aggr[:, 0:1],
                    scalar1=aggr[:, 0:1], scalar2=aggr[:, 1:2],
                    op0=ALU.mult, op1=ALU.add,
                )
                stats2.append(st2)
            gn_stats[s] = stats2

        def alloc_xh8(s):
            xh8[s] = xhpool.tile([128, 2, N], FP8, tag="xh8", name="xh8")

        def emit_gn_combine(s):
            # group combine on PE (tiny matmul; placed right after AV in the
            # PE stream so its DVE/ACT middle hides under the AV matmuls),
            # then group-level mu/rstd on 32 partitions
            stats2 = gn_stats[s]
            gps = ppb.tile([G, 2], F32, tag="big", name="gps")
            for t in range(CT):
                nc.tensor.matmul(gps[:], ag_sb[t][:], stats2[t][:],
                                 start=(t == 0), stop=(t == CT - 1))
            g2 = stpool.tile([G, 2], F32, tag="g2", name="g2")
            nc.vector.tensor_copy(g2[:], gps[:])
            murs = stpool.tile([G, 2], F32, tag="murs", name="murs")
            nc.vector.tensor_copy(murs[:, 0:1], g2[:, 0:1])
            nv = stpool.tile([G, 1], F32, tag="nv", name="nv")
            nc.vector.tensor_scalar(
                out=nv[:], in0=g2[:, 0:1],
                scalar1=g2[:, 0:1], scalar2=g2[:, 1:2],
                op0=ALU.mult, op1=ALU.subtract)
            lnv = stpool.tile([G, 1], F32, tag="lnv", name="lnv")
            nc.scalar.activation(lnv[:], nv[:], AF.Ln,
                                 bias=eps_sb[0:G, :], scale=-1.0)
            nc.scalar.activation(murs[:, 1:2], lnv[:], AF.Exp, scale=-0.5)
            return murs

        def emit_gn_bcast(s, murs):
            # broadcast group (mu, rstd) back to channel partitions with a
            # tiny PE matmul, then xhat on DVE
            alloc_xh8(s)
            for t in range(CT):
                bcps = ppb.tile([128, 2], F32, tag="big", name="bcps")
                nc.tensor.matmul(bcps[:], bg_sb[t][:], murs[:],
                                 start=True, stop=True)
                mubc = stpool.tile([128, 2], F32, tag=f"mubc{t}",
                                   name=f"mubc{t}")
                nc.vector.tensor_copy(mubc[:], bcps[:])
                nc.vector.tensor_scalar(
                    out=xh8[s][:, t, :], in0=x_sb[s][t][:],
                    scalar1=mubc[:, 0:1], scalar2=mubc[:, 1:2],
                    op0=ALU.subtract, op1=ALU.mult,
                )

        def emit_gn_finish_pe(s):
            # prologue-only variant: group combine via PE matmuls (PE is
            # idle before the first projection; ~4us shorter than the DMA
            # round-trips)
            stats2 = gn_stats[s]
            gps = ppb.tile([G, 2], F32, tag="big", name="gps")
            for t in range(CT):
                nc.tensor.matmul(gps[:], ag_sb[t][:], stats2[t][:],
                                 start=(t == 0), stop=(t == CT - 1))
            gst = stpool.tile([G, 2], F32, tag="gst", name="gst")
            nc.vector.tensor_copy(gst[:], gps[:])
            alloc_xh8(s)
            for t in range(CT):
                bcps = ppb.tile([128, 2], F32, tag="big", name="bcps")
                nc.tensor.matmul(bcps[:], bg_sb[t][:], gst[:],
                                 start=True, stop=True)
                mm = stpool.tile([128, 2], F32, tag=f"mm{t}", name=f"mm{t}")
                nc.vector.tensor_copy(mm[:], bcps[:])
                nv = stpool.tile([128, 1], F32, tag=f"nv{t}", name=f"nv{t}")
                nc.vector.tensor_scalar(
                    out=nv[:], in0=mm[:, 0:1],
                    scalar1=mm[:, 0:1], scalar2=mm[:, 1:2],
                    op0=ALU.mult, op1=ALU.subtract)
                lnv = stpool.tile([128, 1], F32, tag=f"lnv{t}",
                                  name=f"lnv{t}")
                nc.scalar.activation(lnv[:], nv[:], AF.Ln,
                                     bias=eps_sb[:], scale=-1.0)
                rs = stpool.tile([128, 1], F32, tag=f"rs{t}", name=f"rs{t}")
                nc.scalar.activation(rs[:], lnv[:], AF.Exp, scale=-0.5)
                nc.vector.tensor_scalar(
                    out=xh8[s][:, t, :], in0=x_sb[s][t][:],
                    scalar1=mm[:, 0:1], scalar2=rs[:],
                    op0=ALU.subtract, op1=ALU.mult,
                )

        def emit_uv(s):
            # U' = (M @ xh)*SM + w1*SM  in fp8 pair layout [128, 2, N];
            # V = (xh^T Wv + bv)*SV     in fp8 pair tiles [128, 2, C]
            u8[s] = upool.tile([128, 2, N], FP8, tag="u8", name="u8")
            for ct in range(CT):
                ps = ppb.tile([128, N], F32, tag="big", name="ps")
                for nch in range(2):
                    nc.tensor.matmul(
                        ps[:, nch * 512:(nch + 1) * 512],
                        mt_sb[:, :, ct * 128:(ct + 1) * 128],
                        xh8[s][:, :, nch * 512:(nch + 1) * 512],
                        start=True, stop=True, perf_mode=DR)
                nc.scalar.activation(
                    u8[s][:, ct, :], ps[:], AF.Identity, bias=w1_sb[ct][:])
            v8[s] = [vpool.tile([128, 2, C], FP8, tag=f"v8_{m2}",
                                name=f"v8_{m2}") for m2 in range(MT // 2)]
            for m2 in range(MT // 2):
                ps = ppb.tile([128, 2 * C], F32, tag="big", name="psv")
                for j in range(2):
                    nc.tensor.matmul(
                        ps[:, j * C:(j + 1) * C],
                        xh8[s][:, :, (2 * m2 + j) * 128:(2 * m2 + j + 1) * 128],
                        wv_sb[:],
                        start=True, stop=True, perf_mode=DR)
                nc.vector.tensor_tensor(
                    out=v8[s][m2][:].rearrange("p a b -> p (a b)"),
                    in0=ps[:], in1=bv_sb[:], op=ALU.add)

        # ---- prologue (x(0) DMA already issued at the very top) ----
        emit_load_x(1)
        emit_gn_stats(0)
        emit_gn_finish_pe(0)
        emit_uv(0)

        def emit_scores(s):
            # scores (transposed, fp8 DoubleRow) + exp -> e8 pair tiles
            e8[s] = [epool.tile([128, 2, N], FP8, tag=f"e8_{m2}",
                                name=f"e8_{m2}") for m2 in range(MT // 2)]
            for mt in range(MT):
                ps = ppb.tile([128, N], F32, tag="big", name="ps")
                for nch in range(2):
                    nc.tensor.matmul(
                        ps[:, nch * 512:(nch + 1) * 512],
                        xh8[s][:, :, mt * 128:(mt + 1) * 128],
                        u8[s][:, :, nch * 512:(nch + 1) * 512],
                        start=True, stop=True, perf_mode=DR)
                nc.scalar.activation(e8[s][mt // 2][:, mt % 2, :], ps[:],
                                     AF.Exp, scale=1.0 / SM)

        e8 = [None] * BS
        emit_scores(0)

        for s in range(BS):
            if s + 1 < BS:
                emit_gn_stats(s + 1)

            # --- colsum (ones lhsT -> broadcast) + hout accumulation ---
            cp = ppb.tile([128, N], F32, tag="big", name="cp")
            hp = [ppb.tile([128, N], F32, tag="big", name="hp")
                  for _ in range(CT)]
            M2 = MT // 2
            for m2 in range(M2):
                for nch in range(2):
                    nc.tensor.matmul(
                        cp[:, nch * 512:(nch + 1) * 512],
                        ones_sb[:],
                        e8[s][m2][:, :, nch * 512:(nch + 1) * 512],
                        start=(m2 == 0), stop=(m2 == M2 - 1),
                        perf_mode=DR)
                for ch in range(CT):
                    for nch in range(2):
                        nc.tensor.matmul(
                            hp[ch][:, nch * 512:(nch + 1) * 512],
                            v8[s][m2][:, :, ch * 128:(ch + 1) * 128],
                            e8[s][m2][:, :, nch * 512:(nch + 1) * 512],
                            start=(m2 == 0), stop=(m2 == M2 - 1),
                            perf_mode=DR)

            # next sample's groupnorm tail + projections keep PE busy while
            # this sample's reciprocal/eviction chain runs on ScalarE/DVE
            if s + 1 < BS:
                murs_next = emit_gn_combine(s + 1)
                emit_gn_bcast(s + 1, murs_next)
                emit_uv(s + 1)

            # --- 1/colsum via exp(-ln()) on ScalarE ---
            lncs = hpool.tile([128, N], F32, tag="lncs", name="lncs")
            rbc = hpool.tile([128, N], F32, tag="rbc", name="rbc")
            nc.scalar.activation(lncs[:], cp[:], AF.Ln)
            nc.scalar.activation(rbc[:], lncs[:], AF.Exp, scale=-1.0)
            # --- hout eviction: unwind SV and normalize; halves first so
            # the output projection can start after two of the four ops ---
            h_sb = [hpool.tile([128, N], BF16, tag=f"h{ch}", name=f"h{ch}")
                    for ch in range(CT)]
            for nch in range(2):
                for ch in range(CT):
                    sl = slice(nch * 512, (nch + 1) * 512)
                    nc.vector.scalar_tensor_tensor(
                        out=h_sb[ch][:, sl], in0=hp[ch][:, sl],
                        scalar=1.0 / SV,
                        in1=rbc[:, sl], op0=ALU.mult, op1=ALU.mult)

            if s + 1 < BS:
                emit_scores(s + 1)

            # --- output projection (bf16) + bias + residual ---
            o_sb = [opool.tile([128, N], F32, tag=f"o{dt}", name=f"o{dt}")
                    for dt in range(CT)]
            fps = [ppb.tile([128, N], F32, tag="big", name="ps")
                   for _ in range(CT)]
            for nch in range(2):
                for dt in range(CT):
                    for ct in range(CT):
                        nc.tensor.matmul(
                            fps[dt][:, nch * 512:(nch + 1) * 512],
                            wo_sb[ct][:, dt * 128:(dt + 1) * 128],
                            h_sb[ct][:, nch * 512:(nch + 1) * 512],
                            start=(ct == 0), stop=(ct == CT - 1))
            for dt in range(CT):
                # out = (psum + bo) + x  in one DVE pass
                nc.vector.scalar_tensor_tensor(
                    out=o_sb[dt][:],
                    in0=fps[dt][:],
                    scalar=bo_sb[dt][:],
                    in1=x_sb[s][dt][:],
                    op0=ALU.add, op1=ALU.add)
            for dt in range(CT):
                eng = nc.sync if dt == 0 else nc.gpsimd
                eng.dma_start(y_d[s, dt * 128:(dt + 1) * 128, :],
                              o_sb[dt][:])
            if s + 2 < BS:
                emit_load_x(s + 2)


_NC_CACHE = {}


def _get_nc():
    if "nc" not in _NC_CACHE:
        _NC_CACHE["nc"] = build_nc()
    return _NC_CACHE["nc"]


def _pair(a):
    """[C, X] -> [128, 2, X] fp8 pair layout with c = 128*j + p."""
    a = np.asarray(a, np.float32)
    return np.ascontiguousarray(
        a.reshape(2, 128, a.shape[1]).transpose(1, 0, 2))


def _fp8(a):
    return np.clip(np.asarray(a, np.float32),
                   -240, 240).astype(ml_dtypes.float8_e4m3)


def make_in_maps(**inputs):
    f32 = np.float32
    x = np.asarray(inputs["x"], f32).reshape(B, C, N)
    Wq = np.asarray(inputs["Wq"], f32)
    Wk = np.asarray(inputs["Wk"], f32)
    Wv = np.asarray(inputs["Wv"], f32)
    Wo = np.asarray(inputs["Wo"], f32)
    bq = np.asarray(inputs["bq"], f32)
    bv = np.asarray(inputs["bv"], f32)
    bo = np.asarray(inputs["bo"], f32)
    gq_s = np.asarray(inputs["gq_s"], f32)
    gq_b = np.asarray(inputs["gq_b"], f32)
    gk_s = np.asarray(inputs["gk_s"], f32)
    gv_s = np.asarray(inputs["gv_s"], f32)
    gv_b = np.asarray(inputs["gv_b"], f32)
    # bk and gk_b only shift scores uniformly along the softmax axis -> cancel

    inv_sqrt_c = float(C) ** -0.5
    Wq_eff = (gq_s[:, None] * Wq) * inv_sqrt_c
    bq_eff = (gq_b @ Wq + bq) * inv_sqrt_c
    Wk_eff = gk_s[:, None] * Wk
    m_t = (Wq_eff @ Wk_eff.T) * SM       # lhsT for U: [c', c], fp8-scaled
    w1 = (Wk_eff @ bq_eff) * SM          # [c]
    Wv_eff = gv_s[:, None] * Wv
    bv_eff = gv_b @ Wv + bv

    bf = ml_dtypes.bfloat16
    ag = np.zeros((C, G), f32)
    bg = np.zeros((G, C), f32)
    for c in range(C):
        ag[c, c // GS] = 1.0 / GS
        bg[c // GS, c] = 1.0

    shared = {
        "mt": _fp8(_pair(m_t)),
        "wv": _fp8(_pair(Wv_eff * SV)),
        "wo": Wo.astype(bf).reshape(CT, 128, C),
        "w1": w1.astype(f32).reshape(CT, 128, 1),
        "bo": bo.reshape(CT, 128, 1),
        "bv_bc": np.tile(bv_eff[None, :] * SV, (128, 2)).astype(f32),
        "ag": np.ascontiguousarray(ag.reshape(CT, 128, G)),
        "bg": np.ascontiguousarray(bg.reshape(G, CT, 128).transpose(1, 0, 2)),
    }
    in_maps = []
    for i in range(NCORES):
        m = dict(shared)
        m["x"] = np.ascontiguousarray(x[i * BS:(i + 1) * BS])
        in_maps.append(m)
    return in_maps


def run_sharded(inputs, trace=False, **kwargs):
    from concourse.bass_utils import run_bass_kernel_spmd
    nc = _get_nc()
    in_maps = make_in_maps(**inputs)
    res = run_bass_kernel_spmd(nc, in_maps, core_ids=list(range(NCORES)),
                               trace=trace, **kwargs)
    outs = [np.asarray(res.results[i]["y"], np.float32) for i in range(NCORES)]
    full = np.concatenate(outs, axis=0).reshape(B, C, H, W)
    return full, res


def kernel(**inputs):
    out, _ = run_sharded(inputs, trace=False)
    return out
